# revision 1
# baseline (speedup 1.0000x reference)
"""Trainium2 Bass kernel for nn_AttentionBlock (B=4, C=512, T=2048, H=8, G=32).

Sharding: 8 cores = 4 batches x 2 head-groups (4 heads each).
Per core (batch b, head-group hg):
  h = GroupNorm32(x_b) * scale + bias                  (bn_stats + mask-matmuls)
  q,k = Wq/Wk (pre-scaled by ch**-0.25, host-transposed) @ h   -> [2 pairs x 128, T] fp16
  vT  = h^T @ WvT_aug  (+bias via K=1 rank-1 matmul; ones-columns appended)
  per (pair, t-chunk): ST[s,t] = k^T q  (2 heads packed via PE row-tiling, K=64)
                       pT = exp(ST)     (no max subtraction: logits are O(1))
                       a_aug = vT_aug^T @ pT  (m=65: 64 v-channels + denominator row)
                       normalize via reciprocal + K=65 broadcast-matmul
  partial = WoT_group^T @ a  -> host sums the two partials per batch + proj_b + x.

All matmul operands fp16 (1 elem/cell/cycle on PE), accumulation fp32 in PSUM.
"""

import math
import os
import sys

import numpy as np

for _p in ("/opt/trn_rl_repo", "/root/.axon_site/_ro/trn_rl_repo"):
    if _p not in sys.path and os.path.isdir(_p):
        sys.path.insert(0, _p)

B, C, T = 4, 512, 2048
H = 8
G = 32
EPS = 1e-5
CH = C // H          # 64 head dim
NCORES = 8
NKT = C // 128       # 4 contraction tiles
NTT = T // 128       # 16 sequence tiles
NTC = T // 512       # 4 t-chunks
QSCALE = 1.0 / math.sqrt(math.sqrt(CH))

_PROG = None
LAST_RESULT = None


def _build_program(timing_repeat=0):
    """timing_repeat=0: the real kernel (DMA inputs, body once).
    timing_repeat=K>0: timing variant — inputs replaced by on-device memsets
    (engine timing is data-independent) and the compute body emitted K times,
    so wall(K1)-wall(K0) isolates pure device execution time."""
    import concourse.bass as bass
    import concourse.tile as tile
    from concourse import mybir
    from concourse.bacc import Bacc

    F32 = mybir.dt.float32
    F16 = mybir.dt.float16
    AF = mybir.ActivationFunctionType
    OP = mybir.AluOpType

    timing = timing_repeat > 0
    nreps = max(1, timing_repeat)

    nc = Bacc(trn_type="TRN2")

    if not timing:
        x_d = nc.dram_tensor("x", [4, 128, T], F32, kind="ExternalInput")
        wq_d = nc.dram_tensor("wq", [128, NKT, 256], F16, kind="ExternalInput")
        wk_d = nc.dram_tensor("wk", [128, NKT, 256], F16, kind="ExternalInput")
        wv_d = nc.dram_tensor("wv", [128, NKT, 260], F16, kind="ExternalInput")
        bv_d = nc.dram_tensor("bv", [1, 260], F16, kind="ExternalInput")
        bqk_d = nc.dram_tensor("bqk", [128, 4], F32, kind="ExternalInput")
        wo_d = nc.dram_tensor("wo", [128, 2, 512], F16, kind="ExternalInput")
        gm_d = nc.dram_tensor("gmask", [128, NKT, G], F32, kind="ExternalInput")
        bm_d = nc.dram_tensor("bmask", [G, NKT, 128], F32, kind="ExternalInput")
        gb_d = nc.dram_tensor("gb", [128, NKT, 2], F32, kind="ExternalInput")
    out_d = nc.dram_tensor("out", [4, 128, T], F32, kind="ExternalOutput")

    with tile.TileContext(nc) as tc:
        with (
            tc.tile_pool(name="singles", bufs=1) as singles,
            tc.tile_pool(name="bufs", bufs=2) as bufs,
        ):
            # ---- persistent SBUF: weights / constants ----
            def load(tile_, src):
                if timing:
                    nc.vector.memset(tile_, 0.01)
                else:
                    nc.sync.dma_start(out=tile_, in_=src)

            wq_sb = singles.tile([128, NKT, 256], F16)
            load(wq_sb, None if timing else wq_d[:, :, :])
            wk_sb = singles.tile([128, NKT, 256], F16)
            load(wk_sb, None if timing else wk_d[:, :, :])
            wv_sb = singles.tile([128, NKT, 260], F16)
            load(wv_sb, None if timing else wv_d[:, :, :])
            bv_sb = singles.tile([1, 260], F16)
            load(bv_sb, None if timing else bv_d[:, :])
            bqk_sb = singles.tile([128, 4], F32)
            load(bqk_sb, None if timing else bqk_d[:, :])
            wo_sb = singles.tile([128, 2, 512], F16)
            load(wo_sb, None if timing else wo_d[:, :, :])
            gm_sb = singles.tile([128, NKT, G], F32)
            load(gm_sb, None if timing else gm_d[:, :, :])
            bm_sb = singles.tile([G, NKT, 128], F32)
            load(bm_sb, None if timing else bm_d[:, :, :])
            gb_sb = singles.tile([128, NKT, 2], F32)
            load(gb_sb, None if timing else gb_d[:, :, :])

            ones1 = singles.tile([1, 128], F16)
            nc.vector.memset(ones1, 1.0)
            onesM = singles.tile([65, 64], F16)
            nc.vector.memset(onesM, 0.0)
            nc.vector.memset(onesM[64:65, :], 1.0)
            rd2A = []
            rd2B = []
            for i in range(2):
                ta = singles.tile([65, 512], F16, name=f"rd2A{i}")
                nc.vector.memset(ta, 0.0)
                rd2A.append(ta)
                tb = singles.tile([65, 512], F16, name=f"rd2B{i}")
                nc.vector.memset(tb, 0.0)
                rd2B.append(tb)
            AB = [singles.tile([128, 2], F32, name=f"ab{i}") for i in range(NKT)]
            grp2 = singles.tile([G, 2], F32)
            eps_sb = singles.tile([G, 1], F32)
            nc.vector.memset(eps_sb, EPS)

            # persistent activations
            x_sb = [singles.tile([128, T], F32, name=f"xt{i}") for i in range(NKT)]
            h_sb = [singles.tile([128, T], F16, name=f"ht{i}") for i in range(NKT)]
            q_sb = [singles.tile([128, T], F16, name=f"qp{p}") for p in range(2)]
            k_sb = [singles.tile([128, T], F16, name=f"kp{p}") for p in range(2)]
            vt_sb = singles.tile([128, NTT, 260], F16)
            a_all = [singles.tile([128, T], F16, name=f"aall{p}") for p in range(2)]

            for i in range(NKT):
                if timing:
                    nc.vector.memset(x_sb[i], 0.5)
                else:
                    nc.sync.dma_start(out=x_sb[i], in_=x_d[i])

            for _rep in range(nreps):
                _phase_body(
                    nc, tc, tile, mybir, singles, bufs,
                    wq_sb, wk_sb, wv_sb, bv_sb, bqk_sb, wo_sb, gm_sb, bm_sb,
                    gb_sb, ones1, onesM, rd2A, rd2B, AB, grp2, eps_sb,
                    x_sb, h_sb, q_sb, k_sb, vt_sb, a_all, out_d,
                )

    nc.finalize()
    return nc


def _phase_body(
    nc, tc, tile, mybir, singles, bufs,
    wq_sb, wk_sb, wv_sb, bv_sb, bqk_sb, wo_sb, gm_sb, bm_sb,
    gb_sb, ones1, onesM, rd2A, rd2B, AB, grp2, eps_sb,
    x_sb, h_sb, q_sb, k_sb, vt_sb, a_all, out_d,
):
    F32 = mybir.dt.float32
    F16 = mybir.dt.float16
    AF = mybir.ActivationFunctionType
    OP = mybir.AluOpType
    if True:
        if True:
            # ================= Phase 1: GroupNorm stats =================
            with (
                tc.tile_pool(name="gnp", bufs=2) as gnp,
                tc.tile_pool(name="gps", bufs=1, space="PSUM") as gps,
            ):
                gs_ps = gps.tile([G, 2], F32, tag="gs")
                for i in range(NKT):
                    st6 = gnp.tile([128, 4, 6], F32, tag="st6")
                    for sg in range(4):
                        nc.vector.bn_stats(
                            out=st6[:, sg, :], in_=x_sb[i][:, sg * 512 : (sg + 1) * 512]
                        )
                    mv = gnp.tile([128, 2], F32, tag="mv")
                    nc.vector.bn_aggr(out=mv, in_=st6)
                    s2 = gnp.tile([128, 2], F32, tag="s2", bufs=4)
                    nc.vector.tensor_copy(out=s2[:, 0:1], in_=mv[:, 0:1])
                    nc.vector.tensor_mul(out=s2[:, 1:2], in0=mv[:, 0:1], in1=mv[:, 0:1])
                    nc.vector.tensor_add(out=s2[:, 1:2], in0=s2[:, 1:2], in1=mv[:, 1:2])
                    # group sums: [G,2] += gmask_i^T @ s2
                    nc.tensor.matmul(
                        gs_ps, gm_sb[:, i, :], s2, start=(i == 0), stop=(i == NKT - 1)
                    )
                gtmp = gnp.tile([G, 2], F32, tag="gt")
                nc.vector.tensor_scalar_mul(out=gtmp, in0=gs_ps, scalar1=1.0 / 16.0)
                var = gnp.tile([G, 1], F32, tag="var")
                nc.vector.tensor_mul(out=var, in0=gtmp[:, 0:1], in1=gtmp[:, 0:1])
                nc.vector.tensor_sub(out=var, in0=gtmp[:, 1:2], in1=var)
                # rstd = exp(-0.5 * ln(var + eps))
                nc.scalar.activation(out=var, in_=var, func=AF.Ln, bias=eps_sb)
                nc.scalar.activation(out=grp2[:, 0:1], in_=var, func=AF.Exp, scale=-0.5)
                nc.vector.tensor_copy(out=grp2[:, 1:2], in_=gtmp[:, 0:1])
                for i in range(NKT):
                    ch_ps = gps.tile([128, 2], F32, tag="ch", bufs=2)
                    nc.tensor.matmul(ch_ps, bm_sb[:, i, :], grp2, start=True, stop=True)
                    # A = rstd_c * gamma ; Bc = beta - mean_c * A
                    nc.vector.tensor_mul(
                        out=AB[i][:, 0:1], in0=ch_ps[:, 0:1], in1=gb_sb[:, i, 0:1]
                    )
                    t1 = gnp.tile([128, 1], F32, tag="t1")
                    nc.vector.tensor_mul(out=t1, in0=ch_ps[:, 1:2], in1=AB[i][:, 0:1])
                    nc.vector.tensor_sub(out=AB[i][:, 1:2], in0=gb_sb[:, i, 1:2], in1=t1)
                for i in range(NKT):
                    nc.vector.tensor_scalar(
                        out=h_sb[i],
                        in0=x_sb[i],
                        scalar1=AB[i][:, 0:1],
                        scalar2=AB[i][:, 1:2],
                        op0=OP.mult,
                        op1=OP.add,
                    )

            # ================= Phase 2: QKV =================
            with tc.tile_pool(name="qps", bufs=1, space="PSUM") as qps:
                for pair in range(2):
                    q_ps = [
                        qps.tile([128, 512], F32, tag="qk", bufs=4, name=f"qps{pair}_{t}")
                        for t in range(NTC)
                    ]
                    for kt in range(NKT):
                        for tcq in range(NTC):
                            nc.tensor.matmul(
                                q_ps[tcq],
                                wq_sb[:, kt, pair * 128 : (pair + 1) * 128],
                                h_sb[kt][:, tcq * 512 : (tcq + 1) * 512],
                                start=(kt == 0),
                                stop=(kt == NKT - 1),
                            )
                    for tcq in range(NTC):
                        nc.vector.tensor_scalar_add(
                            out=q_sb[pair][:, tcq * 512 : (tcq + 1) * 512],
                            in0=q_ps[tcq],
                            scalar1=bqk_sb[:, pair : pair + 1],
                        )
                for pair in range(2):
                    k_ps = [
                        qps.tile([128, 512], F32, tag="qk", bufs=4, name=f"kps{pair}_{t}")
                        for t in range(NTC)
                    ]
                    for kt in range(NKT):
                        for tcq in range(NTC):
                            nc.tensor.matmul(
                                k_ps[tcq],
                                wk_sb[:, kt, pair * 128 : (pair + 1) * 128],
                                h_sb[kt][:, tcq * 512 : (tcq + 1) * 512],
                                start=(kt == 0),
                                stop=(kt == NKT - 1),
                            )
                    for tcq in range(NTC):
                        nc.vector.tensor_scalar_add(
                            out=k_sb[pair][:, tcq * 512 : (tcq + 1) * 512],
                            in0=k_ps[tcq],
                            scalar1=bqk_sb[:, 2 + pair : 3 + pair],
                        )
                # vT (both pairs at once; ones columns filled by the K=1 bias matmul)
                for tt in range(NTT):
                    vt_ps = qps.tile([128, 260], F32, tag="vt", bufs=2)
                    for kt in range(NKT):
                        nc.tensor.matmul(
                            vt_ps,
                            h_sb[kt][:, tt * 128 : (tt + 1) * 128],
                            wv_sb[:, kt, :],
                            start=(kt == 0),
                            stop=False,
                        )
                    nc.tensor.matmul(vt_ps, ones1, bv_sb, start=False, stop=True)
                    nc.vector.tensor_copy(out=vt_sb[:, tt, :], in_=vt_ps)

            # ================= Phase 3: attention =================
            with (
                tc.tile_pool(name="sps", bufs=1, space="PSUM") as sps,
                tc.tile_pool(name="ptp", bufs=1) as ptp,
            ):
                for pair in range(2):
                    for tcn in range(NTC):
                        tcs = slice(tcn * 512, (tcn + 1) * 512)
                        aA_ps = sps.tile([65, 512], F32, tag="aA", bufs=1)
                        aB_ps = sps.tile([65, 512], F32, tag="aB", bufs=1)
                        for blk in range(NTT // 2):
                            ST_A = sps.tile([128, 1024], F32, tag="st", bufs=2)
                            ST_B = sps.tile([128, 1024], F32, tag="st", bufs=2)
                            for j in range(2):
                                sti = blk * 2 + j
                                ss = slice(sti * 128, (sti + 1) * 128)
                                js = slice(j * 512, (j + 1) * 512)
                                nc.tensor.matmul(
                                    ST_A[:, js],
                                    k_sb[pair][0:64, ss],
                                    q_sb[pair][0:64, tcs],
                                    start=True,
                                    stop=True,
                                )
                                nc.tensor.matmul(
                                    ST_B[:, js],
                                    k_sb[pair][64:128, ss],
                                    q_sb[pair][64:128, tcs],
                                    start=True,
                                    stop=True,
                                )
                            pT_A = ptp.tile([128, 1024], F16, tag="pt", bufs=4)
                            pT_B = ptp.tile([128, 1024], F16, tag="pt", bufs=4)
                            nc.scalar.activation(out=pT_A, in_=ST_A, func=AF.Exp)
                            nc.scalar.activation(out=pT_B, in_=ST_B, func=AF.Exp)
                            for j in range(2):
                                sti = blk * 2 + j
                                js = slice(j * 512, (j + 1) * 512)
                                nc.tensor.matmul(
                                    aA_ps,
                                    vt_sb[:, sti, pair * 130 : pair * 130 + 65],
                                    pT_A[:, js],
                                    start=(sti == 0),
                                    stop=(sti == NTT - 1),
                                )
                                nc.tensor.matmul(
                                    aB_ps,
                                    vt_sb[:, sti, pair * 130 + 65 : pair * 130 + 130],
                                    pT_B[:, js],
                                    start=(sti == 0),
                                    stop=(sti == NTT - 1),
                                )
                        # finalize: normalize by the denominator row (index 64)
                        par = tcn % 2
                        with nc.allow_low_precision(reason="fp16 softmax denominators"):
                            nc.vector.reciprocal(
                                out=rd2A[par][64:65, :], in_=aA_ps[64:65, :]
                            )
                            nc.vector.reciprocal(
                                out=rd2B[par][64:65, :], in_=aB_ps[64:65, :]
                            )
                        rdbA = sps.tile([64, 512], F32, tag="rdbA", bufs=1)
                        rdbB = sps.tile([64, 512], F32, tag="rdbB", bufs=1)
                        nc.tensor.matmul(rdbA, onesM, rd2A[par], start=True, stop=True)
                        nc.tensor.matmul(rdbB, onesM, rd2B[par], start=True, stop=True)
                        rdbA_sb = bufs.tile([64, 512], F32, tag="rdsA", bufs=2)
                        rdbB_sb = bufs.tile([64, 512], F32, tag="rdsB", bufs=2)
                        nc.vector.tensor_copy(out=rdbA_sb, in_=rdbA)
                        nc.vector.tensor_copy(out=rdbB_sb, in_=rdbB)
                        nc.vector.tensor_mul(
                            out=a_all[pair][0:64, tcs], in0=aA_ps[0:64, :], in1=rdbA_sb
                        )
                        tmpB = bufs.tile([64, 512], F16, tag="tmpB", bufs=2)
                        nc.vector.tensor_mul(out=tmpB, in0=aB_ps[0:64, :], in1=rdbB_sb)
                        nc.sync.dma_start(out=a_all[pair][64:128, tcs], in_=tmpB)

            # ================= Phase 4: proj partial =================
            with (
                tc.tile_pool(name="pps", bufs=1, space="PSUM") as pps,
                tc.tile_pool(name="outp", bufs=1) as outp,
            ):
                for m in range(4):
                    op_ps = [
                        pps.tile([128, 512], F32, tag="op", bufs=4, name=f"op{m}_{t}")
                        for t in range(NTC)
                    ]
                    for pair in range(2):
                        for tc2 in range(NTC):
                            nc.tensor.matmul(
                                op_ps[tc2],
                                wo_sb[:, pair, m * 128 : (m + 1) * 128],
                                a_all[pair][:, tc2 * 512 : (tc2 + 1) * 512],
                                start=(pair == 0),
                                stop=(pair == 1),
                            )
                    out_sb = outp.tile([128, T], F32, tag="out", bufs=2)
                    for tc2 in range(NTC):
                        nc.vector.tensor_copy(
                            out=out_sb[:, tc2 * 512 : (tc2 + 1) * 512], in_=op_ps[tc2]
                        )
                    nc.sync.dma_start(out=out_d[m], in_=out_sb)


def _get_program(timing_repeat=0):
    global _PROG
    if timing_repeat:
        return _build_program(timing_repeat)
    if _PROG is None:
        _PROG = _build_program()
    return _PROG


def _core_inputs(core, x, norm_scale, norm_bias, qkv_w, qkv_b, proj_w, proj_b):
    b, hg = core // 2, core % 2
    f16 = np.float16
    f32 = np.float32
    hs = slice(hg * 256, hg * 256 + 256)  # head-group channel range

    qw = qkv_w[0:C][hs] * QSCALE          # [256, 512]
    kw = qkv_w[C : 2 * C][hs] * QSCALE
    vw = qkv_w[2 * C : 3 * C][hs]
    qb = qkv_b[0:C][hs] * QSCALE          # [256]
    kb = qkv_b[C : 2 * C][hs] * QSCALE
    vb = qkv_b[2 * C : 3 * C][hs]

    def to_sb_layout(wT):  # [C, 256] -> [128, NKT, 256]
        return np.ascontiguousarray(
            wT.reshape(NKT, 128, 256).transpose(1, 0, 2)
        )

    wq = to_sb_layout(qw.T).astype(f16)
    wk = to_sb_layout(kw.T).astype(f16)

    vwT = vw.T  # [512, 256]
    wv = np.zeros((C, 260), f32)
    bv = np.zeros((1, 260), f32)
    for p in range(2):
        wv[:, p * 130 : p * 130 + 64] = vwT[:, p * 128 : p * 128 + 64]
        wv[:, p * 130 + 65 : p * 130 + 129] = vwT[:, p * 128 + 64 : p * 128 + 128]
        bv[0, p * 130 : p * 130 + 64] = vb[p * 128 : p * 128 + 64]
        bv[0, p * 130 + 64] = 1.0
        bv[0, p * 130 + 65 : p * 130 + 129] = vb[p * 128 + 64 : p * 128 + 128]
        bv[0, p * 130 + 129] = 1.0
    wv = np.ascontiguousarray(wv.reshape(NKT, 128, 260).transpose(1, 0, 2)).astype(f16)
    bv = bv.astype(f16)

    bqk = np.stack(
        [qb[0:128], qb[128:256], kb[0:128], kb[128:256]], axis=1
    ).astype(f32)  # [128, 4]

    woT = proj_w[:, hs].T  # [256, 512]
    wo = np.ascontiguousarray(woT.reshape(2, 128, 512).transpose(1, 0, 2)).astype(f16)

    # GroupNorm masks: channel c (tile i, partition p) belongs to group (i*128+p)//16
    ch_idx = np.arange(C)
    grp_of = ch_idx // 16
    gmask = np.zeros((C, G), f32)
    gmask[ch_idx, grp_of] = 1.0
    gm = np.ascontiguousarray(gmask.reshape(NKT, 128, G).transpose(1, 0, 2))
    bm = np.ascontiguousarray(
        gmask.T.reshape(G, NKT, 128)
    )  # [G, NKT, 128]: bmask[g, i, c]
    gb = np.ascontiguousarray(
        np.stack([norm_scale, norm_bias], axis=1).reshape(NKT, 128, 2).transpose(1, 0, 2)
    ).astype(f32)

    return {
        "x": np.ascontiguousarray(x[b].reshape(NKT, 128, T)).astype(f32),
        "wq": wq,
        "wk": wk,
        "wv": wv,
        "bv": bv,
        "bqk": bqk,
        "wo": wo,
        "gmask": gm,
        "bmask": bm,
        "gb": gb,
    }


def kernel(x, norm_scale, norm_bias, qkv_w, qkv_b, proj_w, proj_b):
    global LAST_RESULT
    x = np.asarray(x, np.float32)
    norm_scale = np.asarray(norm_scale, np.float32)
    norm_bias = np.asarray(norm_bias, np.float32)
    qkv_w = np.asarray(qkv_w, np.float32)
    qkv_b = np.asarray(qkv_b, np.float32)
    proj_w = np.asarray(proj_w, np.float32)
    proj_b = np.asarray(proj_b, np.float32)

    from concourse.bass_utils import run_bass_kernel_spmd

    nc = _get_program()
    in_maps = [
        _core_inputs(c, x, norm_scale, norm_bias, qkv_w, qkv_b, proj_w, proj_b)
        for c in range(NCORES)
    ]
    res = run_bass_kernel_spmd(
        nc,
        in_maps,
        core_ids=list(range(NCORES)),
        trace=bool(int(os.environ.get("KERNEL_TRACE", "0"))),
    )
    LAST_RESULT = res
    out = np.empty((B, C, T), np.float32)
    for b in range(B):
        p0 = res.results[2 * b]["out"].reshape(C, T)
        p1 = res.results[2 * b + 1]["out"].reshape(C, T)
        out[b] = x[b] + proj_b[:, None] + p0 + p1
    return out

